# revision 13
# baseline (speedup 1.0000x reference)
"""Contrastive loss kernel for Trainium2, 8 NeuronCores (SPMD).

Computes mean_i(-log(sum_j exp((z/T)@(z/T).T)_ij / N)) for z [16384,128].

G = zs@zs.T is symmetric: each 128-row tile R computes only col tiles
C = (R+k) mod 128 for k = 0..63, plus a single delta=64 block for R < 64.
Row sums come from ACT accum_out during the exp pass; the transpose
(column) contributions are accumulated into SBUF colacc tiles (copy on
first touch, add after; the last add also writes a bf16 shadow) and
partition-reduced with a bf16 ones-matmul as soon as each 2048-col group
is complete.

Per-core uniformity for SPMD: core c owns row tiles R = 8m + c and gets
zsT rotated left by c*128 cols, making every offset compile-time; the
host un-rotates the colparts output.

colacc is split into 8 independent 2048-col tiles with a static
tile->engine map (5 DVE / 3 GPSIMD) so the two engines' merge chains
never serialize against each other.
"""

import numpy as np
import ml_dtypes

TEMPERATURE = 0.1
N = 16384
D = 128
NCORES = 8
NT = 128
MPC = 16          # bands per core; R = 8m + c
GW = 2048         # colacc group width
NG = N // GW      # 8 groups

_compiled = {}

# gpsimd offload measured as a net loss (its 2-input SBUF ops contend for
# SBUF ports and inflate DVE op latency) -- everything stays on DVE.
_GP_GROUPS = set()


def _schedule():
    """Returns (bands, first_set, last_set, group_ready).

    bands[m] = list of chunks {off, w, merge=[(j, k, t), ...]}
    first/last_set: {(m, k)} merge entries that are the first/last touch
    of their rotated col tile. group_ready[g] = band after which colacc
    group g is final.
    """
    bands = []
    touches = {t: [] for t in range(NT)}
    for m in range(MPC):
        chunks = []
        # delta-64 chunk FIRST: ending a band on a tiny (128-wide) ACT op
        # starves ACT while PE refills psum for the next band (~1.5us/band
        # bubble measured); leading with it hides the refill under the
        # band's last 2048-wide exp.
        if m < 8:
            t64 = (m * 8 + 64) % NT
            chunks.append(dict(off=8192, w=128, merge=[(0, 64, t64)]))
        for ci in range(4):
            off = ci * 2048
            merge = []
            for j in range(16):
                k = ci * 16 + j
                if k == 0:
                    continue  # diag tile: row-part only
                t = (m * 8 + k) % NT
                merge.append((j, k, t))
            chunks.append(dict(off=off, w=2048, merge=merge))
        bands.append(chunks)
        for ch in chunks:
            for (j, k, t) in ch["merge"]:
                touches[t].append((m, k))
    assert all(touches[t] for t in range(NT))
    first_set = {touches[t][0] for t in range(NT)}
    last_set = {touches[t][-1] for t in range(NT)}
    group_ready = {}
    for g in range(NG):
        group_ready[g] = max(
            touches[t][-1][0] for t in range(g * (GW // 128),
                                             (g + 1) * (GW // 128))
        )
    return bands, first_set, last_set, group_ready


def _build():
    import concourse.bacc as bacc
    import concourse.mybir as mybir
    import concourse.tile as tile

    bf16 = mybir.dt.bfloat16
    f32 = mybir.dt.float32

    nc = bacc.Bacc()
    zrot = nc.dram_tensor("zrot", [D, N], bf16, kind="ExternalInput")
    zrows = nc.dram_tensor("zrows", [D, MPC * 128], bf16, kind="ExternalInput")
    out_rows = nc.dram_tensor("rowsums", [128, MPC], f32, kind="ExternalOutput")
    out_cols = nc.dram_tensor("colparts", [1, N], f32, kind="ExternalOutput")

    bands, first_set, last_set, group_ready = _schedule()
    max_chunks = max(len(b) for b in bands)

    with tile.TileContext(nc) as tc:
        with (
            tc.tile_pool(name="persist", bufs=1) as persist,
            tc.tile_pool(name="work", bufs=4) as work,
            tc.tile_pool(name="cstage", bufs=2) as cstage_pool,
            tc.tile_pool(name="psum", bufs=2, space="PSUM") as psum_pool,
        ):
            ZC = 2048
            # band 0's lhsT as its own tiny tile so the first matmul only
            # waits on a 32KB DMA, not the full zrows transfer
            zr0_sb = persist.tile([D, 128], bf16, tag="zr0")
            nc.sync.dma_start(out=zr0_sb, in_=zrows[:, 0:128])
            zr_sb = persist.tile([D, MPC * 128], bf16, tag="zr")
            nc.sync.dma_start(out=zr_sb, in_=zrows[:, :])
            zt_sb = []
            for t8 in range(N // ZC):
                t = persist.tile([D, ZC], bf16, tag=f"zt{t8}")
                nc.sync.dma_start(out=t, in_=zrot[:, t8 * ZC:(t8 + 1) * ZC])
                zt_sb.append(t)

            # bf16 colacc: both TT operands 16-bit enables DVE 2x mode,
            # halving the merge cost. Accumulating ~9 bf16 adds costs
            # ~0.3% on colparts -> ~1e-5 on the final scalar (validated in
            # the 8-core sim, which models tile dtypes).
            colacc = [persist.tile([128, GW], bf16, tag=f"ca{g}",
                                   name=f"ca{g}") for g in range(NG)]
            rsums = persist.tile([128, MPC], f32, tag="rsums")
            ones_sb = persist.tile([128, 1], bf16, tag="ones")
            nc.vector.memset(ones_sb, 1.0)

            def emit_strip(g):
                # partition-reduce colacc_bf[g] -> colparts[g*GW : +GW]
                strip = psum_pool.tile([1, GW], f32, tag="ps")
                for q in range(GW // 512):
                    nc.tensor.matmul(
                        strip[:, q * 512:(q + 1) * 512],
                        ones_sb,
                        colacc[g][:, q * 512:(q + 1) * 512],
                        start=True,
                        stop=True,
                    )
                stage = cstage_pool.tile([1, GW], f32, tag="cstage")
                # tail strips (several groups all complete after the last
                # band) alternate DVE/ACT so the copies pipeline
                if group_ready[g] == MPC - 1 and g % 2 == 1:
                    nc.scalar.copy(stage, strip)
                else:
                    nc.vector.tensor_copy(stage, strip)
                nc.sync.dma_start(
                    out=out_cols[:, g * GW:(g + 1) * GW], in_=stage
                )

            for m in range(MPC):
                S = 1024 * m
                lhsT = zr0_sb if m == 0 else zr_sb[:, m * 128:(m + 1) * 128]
                chunks = bands[m]
                rparts = work.tile([128, max_chunks], f32, tag="rparts")
                for ci, ch in enumerate(chunks):
                    off, w = ch["off"], ch["w"]
                    ps = psum_pool.tile([128, 2048], f32, tag="ps")
                    pos = 0
                    while pos < w:
                        col = (S + off + pos) % N
                        t8 = col // ZC
                        lim = min(512 - pos % 512, w - pos,
                                  (t8 + 1) * ZC - col)
                        nc.tensor.matmul(
                            ps[:, pos:pos + lim],
                            lhsT,
                            zt_sb[t8][:, col - t8 * ZC: col - t8 * ZC + lim],
                            start=True,
                            stop=True,
                        )
                        pos += lim
                    e = work.tile([128, 2048], bf16, tag="scratch")
                    nc.scalar.activation(
                        e[:, :w],
                        ps[:, :w],
                        mybir.ActivationFunctionType.Exp,
                        accum_out=rparts[:, ci:ci + 1],
                    )
                    # merge into colacc: maximal runs of consecutive tiles
                    # sharing (group, fresh, last); groups break runs so
                    # each run lives in one colacc tile / one engine.
                    merge = ch["merge"]
                    i = 0
                    while i < len(merge):
                        j0, k0, t0 = merge[i]
                        g = t0 // (GW // 128)
                        fr = (m, k0) in first_set
                        i2 = i + 1
                        while i2 < len(merge):
                            jj, kk, tt = merge[i2]
                            if (jj != merge[i2 - 1][0] + 1
                                    or tt != merge[i2 - 1][2] + 1
                                    or tt // (GW // 128) != g
                                    or ((m, kk) in first_set) != fr):
                                break
                            i2 += 1
                        width = (i2 - i) * 128
                        src = e[:, j0 * 128: j0 * 128 + width]
                        gcol = t0 * 128 - g * GW
                        dstf = colacc[g][:, gcol:gcol + width]
                        if fr:
                            nc.vector.tensor_copy(dstf, src)
                        else:
                            nc.vector.tensor_add(dstf, dstf, src)
                        i = i2
                nc.vector.reduce_sum(
                    rsums[:, m:m + 1],
                    rparts[:, 0:len(chunks)],
                    axis=mybir.AxisListType.X,
                )
                for g in range(NG):
                    if group_ready[g] == m:
                        emit_strip(g)

            nc.sync.dma_start(out=out_rows[:, :], in_=rsums)
    nc.finalize()
    return nc


def _get_nc():
    if "nc" not in _compiled:
        _compiled["nc"] = _build()
    return _compiled["nc"]


def _make_in_maps(z):
    zs = np.asarray(z, dtype=np.float32) * np.float32(1.0 / TEMPERATURE)
    zsT = np.ascontiguousarray(zs.T).astype(ml_dtypes.bfloat16)
    in_maps = []
    for c in range(NCORES):
        zrot = np.ascontiguousarray(np.roll(zsT, -c * 128, axis=1))
        zrows = np.ascontiguousarray(
            np.concatenate(
                [
                    zsT[:, (8 * m + c) * 128:(8 * m + c + 1) * 128]
                    for m in range(MPC)
                ],
                axis=1,
            )
        )
        in_maps.append({"zrot": zrot, "zrows": zrows})
    return in_maps


def _combine(results):
    rowsum = np.zeros(N, np.float64)
    colsum = np.zeros(N, np.float64)
    for c, r in enumerate(results):
        rs = np.asarray(r["rowsums"])  # [128, MPC]
        for m in range(MPC):
            R = 8 * m + c
            rowsum[R * 128:(R + 1) * 128] += rs[:, m]
        colsum += np.roll(np.asarray(r["colparts"])[0].astype(np.float64),
                          c * 128)
    total = rowsum + colsum
    l = -(np.log(total) - np.log(float(N)))
    return np.float32(l.mean())


def kernel(z: np.ndarray) -> np.ndarray:
    from concourse.bass_utils import run_bass_kernel_spmd

    nc = _get_nc()
    res = run_bass_kernel_spmd(nc, _make_in_maps(z), list(range(NCORES)))
    return _combine(res.results)


# revision 14
# speedup vs baseline: 1.0599x; 1.0599x over previous
"""Contrastive loss kernel for Trainium2 (8 NeuronCores, SPMD row-sharded).

Computes mean_i(-log(sum_j exp((z/T)@(z/T).T)_ij / N)) for z [16384, 128],
T = 0.1 -- HW exec ~183 us across 8 cores.

G = zs@zs.T is symmetric: each 128-row tile R computes only col tiles
C = (R+k) mod 128 for k = 0..63, plus a single delta=64 block for R < 64.
Row sums come from ACT accum_out during the exp pass; the transpose
(column) contributions are accumulated into SBUF colacc tiles (copy on
first touch, add after; the last add also writes a bf16 shadow) and
partition-reduced with a bf16 ones-matmul as soon as each 2048-col group
is complete.

Per-core uniformity for SPMD: core c owns row tiles R = 8m + c and gets
zsT rotated left by c*128 cols, making every offset compile-time; the
host un-rotates the colparts output.

colacc is split into 8 independent 2048-col bf16 tiles: 16-bit operands
enable the DVE 2x mode for the merge adds, and the split keeps strip
reduces independent of unrelated merges.
"""

import numpy as np
import ml_dtypes

TEMPERATURE = 0.1
N = 16384
D = 128
NCORES = 8
NT = 128
MPC = 16          # bands per core; R = 8m + c
GW = 2048         # colacc group width
NG = N // GW      # 8 groups

_compiled = {}

# gpsimd offload measured as a net loss (its 2-input SBUF ops contend for
# SBUF ports and inflate DVE op latency) -- everything stays on DVE.
_GP_GROUPS = set()


def _schedule():
    """Returns (bands, first_set, last_set, group_ready).

    bands[m] = list of chunks {off, w, merge=[(j, k, t), ...]}
    first/last_set: {(m, k)} merge entries that are the first/last touch
    of their rotated col tile. group_ready[g] = band after which colacc
    group g is final.
    """
    bands = []
    touches = {t: [] for t in range(NT)}
    for m in range(MPC):
        chunks = []
        for ci in range(4):
            off = ci * 2048
            merge = []
            for j in range(16):
                k = ci * 16 + j
                if k == 0:
                    continue  # diag tile: row-part only
                t = (m * 8 + k) % NT
                merge.append((j, k, t))
            chunks.append(dict(off=off, w=2048, merge=merge))
        if m < 8:
            t64 = (m * 8 + 64) % NT
            chunks.append(dict(off=8192, w=128, merge=[(0, 64, t64)]))
        bands.append(chunks)
        for ch in chunks:
            for (j, k, t) in ch["merge"]:
                touches[t].append((m, k))
    assert all(touches[t] for t in range(NT))
    first_set = {touches[t][0] for t in range(NT)}
    last_set = {touches[t][-1] for t in range(NT)}
    group_ready = {}
    for g in range(NG):
        group_ready[g] = max(
            touches[t][-1][0] for t in range(g * (GW // 128),
                                             (g + 1) * (GW // 128))
        )
    return bands, first_set, last_set, group_ready


def _build():
    import concourse.bacc as bacc
    import concourse.mybir as mybir
    import concourse.tile as tile

    bf16 = mybir.dt.bfloat16
    f32 = mybir.dt.float32

    nc = bacc.Bacc()
    zrot = nc.dram_tensor("zrot", [D, N], bf16, kind="ExternalInput")
    zrows = nc.dram_tensor("zrows", [D, MPC * 128], bf16, kind="ExternalInput")
    out_rows = nc.dram_tensor("rowsums", [128, MPC], f32, kind="ExternalOutput")
    out_cols = nc.dram_tensor("colparts", [1, N], f32, kind="ExternalOutput")

    bands, first_set, last_set, group_ready = _schedule()
    max_chunks = max(len(b) for b in bands)

    with tile.TileContext(nc) as tc:
        with (
            tc.tile_pool(name="persist", bufs=1) as persist,
            tc.tile_pool(name="work", bufs=4) as work,
            tc.tile_pool(name="cstage", bufs=2) as cstage_pool,
            tc.tile_pool(name="psum", bufs=2, space="PSUM") as psum_pool,
        ):
            ZC = 2048
            zr_sb = persist.tile([D, MPC * 128], bf16, tag="zr")
            nc.sync.dma_start(out=zr_sb, in_=zrows[:, :])
            zt_sb = []
            for t8 in range(N // ZC):
                t = persist.tile([D, ZC], bf16, tag=f"zt{t8}")
                nc.sync.dma_start(out=t, in_=zrot[:, t8 * ZC:(t8 + 1) * ZC])
                zt_sb.append(t)

            # bf16 colacc: both TT operands 16-bit enables DVE 2x mode,
            # halving the merge cost. Accumulating ~9 bf16 adds costs
            # ~0.3% on colparts -> ~1e-5 on the final scalar (validated in
            # the 8-core sim, which models tile dtypes).
            colacc = [persist.tile([128, GW], bf16, tag=f"ca{g}",
                                   name=f"ca{g}") for g in range(NG)]
            rsums = persist.tile([128, MPC], f32, tag="rsums")
            ones_sb = persist.tile([128, 1], bf16, tag="ones")
            nc.vector.memset(ones_sb, 1.0)

            def emit_strip(g):
                # partition-reduce colacc_bf[g] -> colparts[g*GW : +GW]
                strip = psum_pool.tile([1, GW], f32, tag="ps")
                for q in range(GW // 512):
                    nc.tensor.matmul(
                        strip[:, q * 512:(q + 1) * 512],
                        ones_sb,
                        colacc[g][:, q * 512:(q + 1) * 512],
                        start=True,
                        stop=True,
                    )
                stage = cstage_pool.tile([1, GW], f32, tag="cstage")
                nc.vector.tensor_copy(stage, strip)
                nc.sync.dma_start(
                    out=out_cols[:, g * GW:(g + 1) * GW], in_=stage
                )

            for m in range(MPC):
                S = 1024 * m
                lhsT = zr_sb[:, m * 128:(m + 1) * 128]
                chunks = bands[m]
                rparts = work.tile([128, max_chunks], f32, tag="rparts")
                for ci, ch in enumerate(chunks):
                    off, w = ch["off"], ch["w"]
                    ps = psum_pool.tile([128, 2048], f32, tag="ps")
                    pos = 0
                    while pos < w:
                        col = (S + off + pos) % N
                        t8 = col // ZC
                        lim = min(512 - pos % 512, w - pos,
                                  (t8 + 1) * ZC - col)
                        nc.tensor.matmul(
                            ps[:, pos:pos + lim],
                            lhsT,
                            zt_sb[t8][:, col - t8 * ZC: col - t8 * ZC + lim],
                            start=True,
                            stop=True,
                        )
                        pos += lim
                    e = work.tile([128, 2048], bf16, tag="scratch")
                    nc.scalar.activation(
                        e[:, :w],
                        ps[:, :w],
                        mybir.ActivationFunctionType.Exp,
                        accum_out=rparts[:, ci:ci + 1],
                    )
                    # merge into colacc: maximal runs of consecutive tiles
                    # sharing (group, fresh, last); groups break runs so
                    # each run lives in one colacc tile / one engine.
                    merge = ch["merge"]
                    i = 0
                    while i < len(merge):
                        j0, k0, t0 = merge[i]
                        g = t0 // (GW // 128)
                        fr = (m, k0) in first_set
                        i2 = i + 1
                        while i2 < len(merge):
                            jj, kk, tt = merge[i2]
                            if (jj != merge[i2 - 1][0] + 1
                                    or tt != merge[i2 - 1][2] + 1
                                    or tt // (GW // 128) != g
                                    or ((m, kk) in first_set) != fr):
                                break
                            i2 += 1
                        width = (i2 - i) * 128
                        src = e[:, j0 * 128: j0 * 128 + width]
                        gcol = t0 * 128 - g * GW
                        dstf = colacc[g][:, gcol:gcol + width]
                        if fr:
                            nc.vector.tensor_copy(dstf, src)
                        else:
                            nc.vector.tensor_add(dstf, dstf, src)
                        i = i2
                nc.vector.reduce_sum(
                    rsums[:, m:m + 1],
                    rparts[:, 0:len(chunks)],
                    axis=mybir.AxisListType.X,
                )
                for g in range(NG):
                    if group_ready[g] == m:
                        emit_strip(g)

            nc.sync.dma_start(out=out_rows[:, :], in_=rsums)
    nc.finalize()
    return nc


def _get_nc():
    if "nc" not in _compiled:
        _compiled["nc"] = _build()
    return _compiled["nc"]


def _make_in_maps(z):
    zs = np.asarray(z, dtype=np.float32) * np.float32(1.0 / TEMPERATURE)
    zsT = np.ascontiguousarray(zs.T).astype(ml_dtypes.bfloat16)
    in_maps = []
    for c in range(NCORES):
        zrot = np.ascontiguousarray(np.roll(zsT, -c * 128, axis=1))
        zrows = np.ascontiguousarray(
            np.concatenate(
                [
                    zsT[:, (8 * m + c) * 128:(8 * m + c + 1) * 128]
                    for m in range(MPC)
                ],
                axis=1,
            )
        )
        in_maps.append({"zrot": zrot, "zrows": zrows})
    return in_maps


def _combine(results):
    rowsum = np.zeros(N, np.float64)
    colsum = np.zeros(N, np.float64)
    for c, r in enumerate(results):
        rs = np.asarray(r["rowsums"])  # [128, MPC]
        for m in range(MPC):
            R = 8 * m + c
            rowsum[R * 128:(R + 1) * 128] += rs[:, m]
        colsum += np.roll(np.asarray(r["colparts"])[0].astype(np.float64),
                          c * 128)
    total = rowsum + colsum
    l = -(np.log(total) - np.log(float(N)))
    return np.float32(l.mean())


def kernel(z: np.ndarray) -> np.ndarray:
    from concourse.bass_utils import run_bass_kernel_spmd

    nc = _get_nc()
    res = run_bass_kernel_spmd(nc, _make_in_maps(z), list(range(NCORES)))
    return _combine(res.results)
